# revision 1
# baseline (speedup 1.0000x reference)
"""DBN-Sigma whitening (group-wise decorrelated batch norm) on 8 trn2 cores.

Strategy (data-parallel over batch N, hint-conformant):
  Pass A (device): each core takes 8 of 64 images; computes per-channel
    sums S1 and the two diagonal 128x128 blocks of the raw second moment
    S2 = sum_m x x^T (only those cover the 16 per-group 16x16 sigmas).
    x is cast once to bf16 (ACT engine, fused row-sum via accum_out);
    m-chunks are transposed to [m, c] layout either on the PE (bf16
    transpose -> bf16 PSUM -> DVE copy) or via the DMA xbar
    (dma_start_transpose, 3D out) -- split tuned so PE and DMA balance;
    cov accumulates over all chunks in PSUM via bf16 matmuls.
  Host: reduce partials over cores (f64), sigma_g = S2_g/m - mean mean^T
    + eps I per 16-channel group, eigh -> wm_g = sigma_g^{-1/2}; fold
    mean subtraction and weight/bias into a per-channel affine.
  Pass B (device, pure f32): out = scale_c * (wm @ x)_c + shift_c,
    streamed with 2-image DMAs; affine applied on the scalar engine
    during the PSUM->SBUF move.

Layout: X [64, 256, 56*56] f32; channels on SBUF partitions (2 halves
of 128), free dim = pixel index m. Per-core m = 8*3136; image pairs
give 6272 = 49*128 exactly (no remainder chunks).
"""

import numpy as np
import ml_dtypes
import concourse.bass as bass
import concourse.bacc as bacc
import concourse.mybir as mybir
import concourse.tile as tile
from concourse.bass_utils import run_bass_kernel_spmd

N_CORES = 8
N, C, H, W = 64, 256, 56, 56
HW = H * W                     # 3136
NL = N // N_CORES              # 8 images per core
G, CG = 16, 16
EPS = 1e-3
M_TOT = N * HW
FP = mybir.dt.float32
BF = mybir.dt.bfloat16

NP_ = NL // 2                  # 4 image pairs per core
FPAIR = 2 * HW                 # 6272 free elems per (pair, half)
NCH = FPAIR // 128             # 49 m-chunks per (pair, half)

# Which of the 8 (pair, half) units route their transposes through the
# DMA xbar instead of the PE (balances PE vs DMA time in pass A).
DMA_T_UNITS = {2, 5}


def _build_pass_a():
    nc = bacc.Bacc("TRN2", target_bir_lowering=False, debug=False,
                   num_devices=N_CORES)
    X_d = nc.dram_tensor("X", [NL, C, HW], BF, kind="ExternalInput")
    eye_d = nc.dram_tensor("eye", [128, 128], BF, kind="ExternalInput")
    S1_d = nc.dram_tensor("S1", [128, 2], FP, kind="ExternalOutput")
    S2_d = nc.dram_tensor("S2", [2, 128, 128], FP, kind="ExternalOutput")
    X = X_d.ap()

    with tile.TileContext(nc) as tc:
        with (
            tc.tile_pool(name="const", bufs=1) as constp,
            tc.tile_pool(name="xbf", bufs=4) as xbp,
            tc.tile_pool(name="xbt", bufs=2) as xbtp,
            tc.tile_pool(name="xtq", bufs=6) as xtqp,
            tc.tile_pool(name="red", bufs=2) as redp,
            tc.tile_pool(name="acc", bufs=1) as accp,
            tc.tile_pool(name="ptp", bufs=4, space="PSUM") as ptp,
            tc.tile_pool(name="cov", bufs=1, space="PSUM") as covp,
        ):
            eye = constp.tile([128, 128], BF)
            nc.sync.dma_start(eye[:], eye_d.ap())
            s1 = accp.tile([128, 2], FP)
            nc.vector.memset(s1[:], 0.0)
            cov = [covp.tile([128, 128], FP, tag=f"cov{h}", name=f"cov{h}")
                   for h in (0, 1)]
            started = [False, False]

            for p in range(NP_):
                for h in (0, 1):
                    u = p * 2 + h
                    xb = xbp.tile([128, FPAIR], BF, tag="xb")
                    for i in (0, 1):
                        nc.sync.dma_start(
                            xb[:, HW * i:HW * (i + 1)],
                            X[2 * p + i, 128 * h:128 * (h + 1), :])
                    r = redp.tile([128, 1], FP, tag="r")
                    scr = redp.tile([128, FPAIR], BF, tag="scr", bufs=1)
                    nc.scalar.activation(scr[:], xb[:],
                                         mybir.ActivationFunctionType.Copy,
                                         accum_out=r[:])
                    nc.vector.tensor_add(s1[:, h:h + 1], s1[:, h:h + 1], r[:])

                    last_u = (p == NP_ - 1)
                    if u in DMA_T_UNITS:
                        xbT = xbtp.tile([128, NCH, 128], BF, tag="xbT")
                        nc.sync.dma_start_transpose(xbT[:], xb[:])
                        for j in range(NCH):
                            sl = xbT[:, j, :]
                            nc.tensor.matmul(
                                cov[h][:], sl, sl,
                                start=not started[h],
                                stop=last_u and j == NCH - 1,
                                skip_group_check=True)
                            started[h] = True
                    else:
                        for q in range(13):        # 49 = 12*4 + 1 chunks
                            nch = 4 if q < 12 else 1
                            pt = ptp.tile([128, nch * 128], BF, tag="pt")
                            for jj in range(nch):
                                m0 = 128 * (4 * q + jj)
                                nc.tensor.transpose(
                                    pt[:, 128 * jj:128 * (jj + 1)],
                                    xb[:, m0:m0 + 128], eye[:])
                            xtq = xtqp.tile([128, nch * 128], BF, tag="xtq")
                            nc.vector.tensor_copy(xtq[:], pt[:])
                            for jj in range(nch):
                                sl = xtq[:, 128 * jj:128 * (jj + 1)]
                                nc.tensor.matmul(
                                    cov[h][:], sl, sl,
                                    start=not started[h],
                                    stop=(last_u and q == 12 and jj == nch - 1),
                                    skip_group_check=True)
                                started[h] = True

            s2sb = accp.tile([128, 256], FP)
            for h in (0, 1):
                nc.vector.tensor_copy(s2sb[:, 128 * h:128 * (h + 1)], cov[h][:])
                nc.sync.dma_start(S2_d.ap()[h], s2sb[:, 128 * h:128 * (h + 1)])
            nc.sync.dma_start(S1_d.ap(), s1[:])

    nc.compile()
    return nc


def _build_pass_b():
    nc = bacc.Bacc("TRN2", target_bir_lowering=False, debug=False,
                   num_devices=N_CORES)
    X_d = nc.dram_tensor("X", [NL, C, HW], FP, kind="ExternalInput")
    wm_d = nc.dram_tensor("wm", [128, 256], FP, kind="ExternalInput")
    sc_d = nc.dram_tensor("sc", [128, 2], FP, kind="ExternalInput")
    sh_d = nc.dram_tensor("sh", [128, 2], FP, kind="ExternalInput")
    Xn_d = nc.dram_tensor("Xn", [NL, C, HW], FP, kind="ExternalOutput")
    X = X_d.ap()
    Xn = Xn_d.ap()

    KT = 448                   # matmul free-dim tile (14 * 448 = 6272)
    NK = FPAIR // KT

    with tile.TileContext(nc) as tc:
        with (
            tc.tile_pool(name="const", bufs=1) as constp,
            tc.tile_pool(name="xin", bufs=3) as xp,
            tc.tile_pool(name="xout", bufs=3) as op,
            tc.tile_pool(name="ps", bufs=4, space="PSUM") as psp,
        ):
            wm = constp.tile([128, 256], FP)
            nc.sync.dma_start(wm[:], wm_d.ap())
            sc = constp.tile([128, 2], FP)
            nc.sync.dma_start(sc[:], sc_d.ap())
            sh = constp.tile([128, 2], FP)
            nc.sync.dma_start(sh[:], sh_d.ap())

            for h in (0, 1):
                for p in range(NP_):
                    xf = xp.tile([128, FPAIR], FP, tag="x")
                    for i in (0, 1):
                        nc.sync.dma_start(
                            xf[:, HW * i:HW * (i + 1)],
                            X[2 * p + i, 128 * h:128 * (h + 1), :])
                    ot = op.tile([128, FPAIR], FP, tag="o")
                    for k in range(NK):
                        ps = psp.tile([128, KT], FP, tag="ps")
                        nc.tensor.matmul(
                            ps[:], wm[:, 128 * h:128 * (h + 1)],
                            xf[:, KT * k:KT * (k + 1)])
                        nc.scalar.activation(
                            ot[:, KT * k:KT * (k + 1)], ps[:],
                            mybir.ActivationFunctionType.Identity,
                            bias=sh[:, h:h + 1], scale=sc[:, h:h + 1])
                    for i in (0, 1):
                        nc.sync.dma_start(
                            Xn[2 * p + i, 128 * h:128 * (h + 1), :],
                            ot[:, HW * i:HW * (i + 1)])

    nc.compile()
    return nc


_PROGS = {}


def _programs():
    if "a" not in _PROGS:
        _PROGS["a"] = _build_pass_a()
        _PROGS["b"] = _build_pass_b()
    return _PROGS["a"], _PROGS["b"]


def kernel(X, weight, bias, _return_results=False):
    X = np.asarray(X, dtype=np.float32)
    weight = np.asarray(weight, dtype=np.float32).reshape(C)
    bias = np.asarray(bias, dtype=np.float32).reshape(C)
    nc_a, nc_b = _programs()

    Xr = X.reshape(N, C, HW)
    shards = [Xr[NL * i:NL * (i + 1)] for i in range(N_CORES)]
    shards_bf = [s.astype(ml_dtypes.bfloat16) for s in shards]
    eye = np.eye(128, dtype=ml_dtypes.bfloat16)
    core_ids = list(range(N_CORES))

    res_a = run_bass_kernel_spmd(
        nc_a, [{"X": s, "eye": eye} for s in shards_bf], core_ids)

    # host reduction of the tiny per-core stats (f64 for cleanliness)
    s1 = np.zeros((128, 2), np.float64)
    s2 = np.zeros((2, 128, 128), np.float64)
    for r in res_a.results:
        s1 += r["S1"].astype(np.float64)
        s2 += r["S2"].astype(np.float64)

    mean = np.concatenate([s1[:, 0], s1[:, 1]]) / M_TOT          # [256]
    wm_bd = np.zeros((2, 128, 128), np.float64)
    for g in range(G):
        h, o = divmod(g, 128 // CG)
        o *= CG
        mg = mean[CG * g:CG * (g + 1)]
        sg = (s2[h][o:o + CG, o:o + CG] / M_TOT - np.outer(mg, mg)
              + EPS * np.eye(CG))
        lam, u = np.linalg.eigh(sg)
        wm_bd[h][o:o + CG, o:o + CG] = (u / np.sqrt(lam)) @ u.T

    wm_full = np.zeros((C, C), np.float64)
    wm_full[:128, :128] = wm_bd[0]
    wm_full[128:, 128:] = wm_bd[1]
    v = wm_full @ mean                                           # [256]
    scale = weight.astype(np.float64)
    shift = bias.astype(np.float64) - scale * v

    wm_in = np.concatenate([wm_bd[0], wm_bd[1]], axis=1).astype(np.float32)
    sc_in = np.stack([scale[:128], scale[128:]], axis=1).astype(np.float32)
    sh_in = np.stack([shift[:128], shift[128:]], axis=1).astype(np.float32)

    res_b = run_bass_kernel_spmd(
        nc_b,
        [{"X": s, "wm": wm_in, "sc": sc_in, "sh": sh_in} for s in shards],
        core_ids)

    out = np.concatenate([r["Xn"] for r in res_b.results], axis=0)
    out = out.reshape(N, C, H, W).astype(np.float32)
    if _return_results:
        return out, (res_a, res_b)
    return out



# revision 6
# speedup vs baseline: 1.1932x; 1.1932x over previous
"""DBN-Sigma whitening (group-wise decorrelated batch norm) on 8 trn2 cores.

Single-pass design (data-parallel over batch N, hint-conformant):
  The per-core shard (8 of 64 images) is cast to bf16 on the host and
  streamed ONCE into SBUF where it stays resident (98 KiB/partition).
  While loading, each (image-pair, channel-half) unit contributes
  per-channel row sums (split ACT/DVE) and the two 128x128 diagonal
  blocks of the raw second moment, accumulated in PSUM via PE
  transposes + bf16 matmuls.
  The tiny stats block [128, 258] f32 (two cov blocks + row sums) is
  AllReduce'd across the 8 cores through DRAM bounce buffers.
  sigma = cov/M - mu mu^T + eps I (masked to the 16x16 group blocks)
  is inverted-square-rooted ON DEVICE with 4 coupled Newton-Schulz
  iterations (f32 PE matmuls; sigma ~ I so NS converges quadratically,
  ||I-sigma|| ~ 2e-2 -> 1e-7 after 3 iters). weight/bias and the mean
  fold into a per-channel affine: out = w * (wm @ x) + (b - w*(wm@mu)).
  Whitening then runs out of the resident SBUF copy (bf16 matmuls,
  PSUM evacuated by ACT/DVE alternating with the affine fused), and
  the bf16 output is upcast to f32 on the host.

HBM traffic per core: 12.85 MB in + 12.85 MB out (vs 64 MB for the
two-pass f32 baseline).
"""

import numpy as np
import ml_dtypes
import concourse.bass as bass
import concourse.bacc as bacc
import concourse.mybir as mybir
import concourse.tile as tile
from concourse.bass_utils import run_bass_kernel_spmd

N_CORES = 8
N, C, H, W = 64, 256, 56, 56
HW = H * W                     # 3136
NL = N // N_CORES              # 8 images per core
G, CG = 16, 16
EPS = 1e-3
M_TOT = N * HW                 # 200704
FP = mybir.dt.float32
BF = mybir.dt.bfloat16

NP_ = NL // 2                  # 4 image pairs per core
NU = 2 * NP_                   # 8 (pair, half) units
FPAIR = 2 * HW                 # 6272 free elems per (pair, half)
NCH = FPAIR // 128             # 49 m-chunks per unit
KT = 448                       # whiten matmul free-dim tile (14 * 448 = 6272)
NKW = FPAIR // KT              # 14
NS_ITERS = 4
ACT_ROWSUM_UNITS = {0, 1, 2, 4, 5, 6}   # rest use DVE tensor_reduce
AF = mybir.ActivationFunctionType


def _build():
    nc = bacc.Bacc("TRN2", target_bir_lowering=False, debug=False,
                   num_devices=N_CORES)
    X_d = nc.dram_tensor("X", [NL, C, HW], BF, kind="ExternalInput")
    eyebf_d = nc.dram_tensor("eyebf", [128, 128], BF, kind="ExternalInput")
    eyef_d = nc.dram_tensor("eyef", [128, 128], FP, kind="ExternalInput")
    mask_d = nc.dram_tensor("mask", [128, 128], FP, kind="ExternalInput")
    epseye_d = nc.dram_tensor("epseye", [128, 128], FP, kind="ExternalInput")
    eye15_d = nc.dram_tensor("eye15", [128, 128], FP, kind="ExternalInput")
    w_d = nc.dram_tensor("wcol", [128, 2], FP, kind="ExternalInput")
    b_d = nc.dram_tensor("bcol", [128, 2], FP, kind="ExternalInput")
    Xn_d = nc.dram_tensor("Xn", [NL, C, HW], BF, kind="ExternalOutput")
    X = X_d.ap()
    Xn = Xn_d.ap()

    with tile.TileContext(nc) as tc:
        with (
            tc.tile_pool(name="const", bufs=1) as constp,
            tc.tile_pool(name="xres", bufs=1) as xresp,
            tc.tile_pool(name="scr", bufs=2) as scrp,
            tc.tile_pool(name="xtq", bufs=6) as xtqp,
            tc.tile_pool(name="stats", bufs=1) as statp,
            tc.tile_pool(name="small", bufs=1) as smallp,
            tc.tile_pool(name="out", bufs=3) as outp,
            tc.tile_pool(name="dram", bufs=2, space="DRAM") as dramp,
        ):
            eyebf = constp.tile([128, 128], BF, tag="eyebf")
            nc.sync.dma_start(eyebf[:], eyebf_d.ap())
            eyef = constp.tile([128, 128], FP, tag="eyef")
            nc.sync.dma_start(eyef[:], eyef_d.ap())
            mask = constp.tile([128, 128], FP, tag="mask")
            nc.sync.dma_start(mask[:], mask_d.ap())
            epseye = constp.tile([128, 128], FP, tag="epseye")
            nc.sync.dma_start(epseye[:], epseye_d.ap())
            eye15 = constp.tile([128, 128], FP, tag="eye15")
            nc.sync.dma_start(eye15[:], eye15_d.ap())
            wcol = constp.tile([128, 2], FP, tag="wcol")
            nc.sync.dma_start(wcol[:], w_d.ap())
            bcol = constp.tile([128, 2], FP, tag="bcol")
            nc.sync.dma_start(bcol[:], b_d.ap())

            xall = xresp.tile([128, NU, FPAIR], BF)       # resident shard
            rs = statp.tile([128, 2, NP_], FP)            # per-unit row sums
            cov_sb = statp.tile([128, 258], FP)           # packed local stats
            red_sb = statp.tile([128, 258], FP)           # allreduced stats

            # ---- phase 1: load resident + stats -------------------------
            with (
                tc.tile_pool(name="ptp", bufs=4, space="PSUM") as ptp,
                tc.tile_pool(name="covp", bufs=1, space="PSUM") as covp,
            ):
                cov = [covp.tile([128, 128], FP, tag=f"cov{h}", name=f"cov{h}")
                       for h in (0, 1)]
                started = [False, False]
                for p in range(NP_):
                    for h in (0, 1):
                        u = 2 * p + h
                        for i in (0, 1):
                            nc.sync.dma_start(
                                xall[:, u, HW * i:HW * (i + 1)],
                                X[2 * p + i, 128 * h:128 * (h + 1), :])
                        xu = xall[:, u, :]
                        if u in ACT_ROWSUM_UNITS:
                            scr = scrp.tile([128, FPAIR], BF, tag="scr")
                            nc.scalar.activation(scr[:], xu, AF.Copy,
                                                 accum_out=rs[:, h, p:p + 1])
                        else:
                            nc.vector.tensor_reduce(
                                rs[:, h, p:p + 1], xu,
                                axis=mybir.AxisListType.X,
                                op=mybir.AluOpType.add)
                        last_u = (p == NP_ - 1)
                        for q in range(13):        # 49 = 12*4 + 1 chunks
                            nch = 4 if q < 12 else 1
                            pt = ptp.tile([128, nch * 128], BF, tag="pt")
                            for jj in range(nch):
                                m0 = 128 * (4 * q + jj)
                                nc.tensor.transpose(
                                    pt[:, 128 * jj:128 * (jj + 1)],
                                    xall[:, u, m0:m0 + 128], eyebf[:])
                            xtq = xtqp.tile([128, nch * 128], BF, tag="xtq")
                            nc.vector.tensor_copy(xtq[:], pt[:])
                            for jj in range(nch):
                                sl = xtq[:, 128 * jj:128 * (jj + 1)]
                                nc.tensor.matmul(
                                    cov[h][:], sl, sl,
                                    start=not started[h],
                                    stop=(last_u and q == 12 and jj == nch - 1),
                                    skip_group_check=True)
                                started[h] = True
                for h in (0, 1):
                    nc.vector.tensor_copy(cov_sb[:, 128 * h:128 * (h + 1)],
                                          cov[h][:])
                    nc.vector.tensor_reduce(cov_sb[:, 256 + h:257 + h],
                                            rs[:, h, :],
                                            axis=mybir.AxisListType.X,
                                            op=mybir.AluOpType.add)

            # ---- collective: sum stats over the 8 cores -----------------
            st_in = dramp.tile([128, 258], FP, tag="st_in")
            st_out = dramp.tile([128, 258], FP, tag="st_out")
            nc.sync.dma_start(st_in[:], cov_sb[:])
            nc.gpsimd.collective_compute(
                "AllReduce", mybir.AluOpType.add,
                replica_groups=[list(range(N_CORES))],
                ins=[st_in.opt()], outs=[st_out.opt()])
            nc.sync.dma_start(red_sb[:], st_out[:])

            # ---- phase 2: sigma -> wm = sigma^(-1/2) on device ----------
            wmbf = smallp.tile([128, 256], BF, tag="wmbf")
            shift = smallp.tile([128, 2], FP, tag="shift")
            with tc.tile_pool(name="ps2", bufs=2, space="PSUM") as ps2p:
                meanc = smallp.tile([128, 2], FP, tag="meanc")
                nc.vector.tensor_scalar_mul(meanc[:], red_sb[:, 256:258],
                                            1.0 / M_TOT)
                meanT = []
                for h in (0, 1):
                    s1pad = smallp.tile([128, 128], FP, tag=f"s1pad{h}",
                                        name=f"s1pad{h}")
                    nc.vector.memset(s1pad[:], 0.0)
                    nc.vector.tensor_copy(s1pad[:, 0:1], meanc[:, h:h + 1])
                    psT = ps2p.tile([128, 128], FP, tag="ps2s")
                    nc.tensor.transpose(psT[:], s1pad[:], eyef[:])
                    mT = smallp.tile([128, 128], FP, tag=f"meanT{h}",
                                     name=f"meanT{h}")
                    nc.vector.tensor_copy(mT[0:1, :], psT[0:1, :])
                    meanT.append(mT)

                Yt = [smallp.tile([128, 128], FP, tag=f"Y{h}", name=f"Y{h}")
                      for h in (0, 1)]
                Zt = [smallp.tile([128, 128], FP, tag=f"Z{h}", name=f"Z{h}")
                      for h in (0, 1)]
                Tt = [smallp.tile([128, 128], FP, tag=f"T{h}", name=f"T{h}")
                      for h in (0, 1)]
                for h in (0, 1):
                    po = ps2p.tile([128, 128], FP, tag="ps2s")
                    nc.tensor.matmul(po[:], meanT[h][0:1, :],
                                     meanT[h][0:1, :], start=True, stop=True)
                    tmp = smallp.tile([128, 128], FP, tag=f"tmp{h}")
                    nc.vector.tensor_scalar_mul(
                        tmp[:], red_sb[:, 128 * h:128 * (h + 1)], 1.0 / M_TOT)
                    nc.vector.tensor_sub(tmp[:], tmp[:], po[:])
                    nc.vector.tensor_mul(tmp[:], tmp[:], mask[:])
                    nc.vector.tensor_add(Yt[h][:], tmp[:], epseye[:])
                    nc.vector.tensor_copy(Zt[h][:], eyef[:])
                # Newton-Schulz: T = 1.5 I - 0.5 Z Y ; Y <- Y T ; Z <- T Z
                # (all iterates are polynomials in sigma -> symmetric, so
                #  matmul's lhsT-transpose is a no-op)
                for it in range(NS_ITERS):
                    for h in (0, 1):
                        pzy = ps2p.tile([128, 128], FP, tag="ps2s")
                        nc.tensor.matmul(pzy[:], Zt[h][:], Yt[h][:],
                                         start=True, stop=True)
                        nc.vector.tensor_scalar_mul(Tt[h][:], pzy[:], -0.5)
                        nc.vector.tensor_add(Tt[h][:], Tt[h][:], eye15[:])
                        if it < NS_ITERS - 1:
                            pyy = ps2p.tile([128, 128], FP, tag="ps2s")
                            nc.tensor.matmul(pyy[:], Yt[h][:], Tt[h][:],
                                             start=True, stop=True)
                            nc.vector.tensor_copy(Yt[h][:], pyy[:])
                        pzz = ps2p.tile([128, 128], FP, tag="ps2s")
                        nc.tensor.matmul(pzz[:], Tt[h][:], Zt[h][:],
                                         start=True, stop=True)
                        nc.vector.tensor_copy(Zt[h][:], pzz[:])
                for h in (0, 1):
                    nc.vector.tensor_copy(wmbf[:, 128 * h:128 * (h + 1)],
                                          Zt[h][:])
                    pm = ps2p.tile([128, 1], FP, tag="pm")
                    nc.tensor.matmul(pm[:], Zt[h][:], meanc[:, h:h + 1],
                                     start=True, stop=True)
                    nc.vector.tensor_mul(shift[:, h:h + 1], wcol[:, h:h + 1],
                                         pm[:])
                    nc.vector.tensor_sub(shift[:, h:h + 1], bcol[:, h:h + 1],
                                         shift[:, h:h + 1])

            # ---- phase 3: whiten from resident SBUF ---------------------
            with tc.tile_pool(name="psw", bufs=6, space="PSUM") as pswp:
                for p in range(NP_):
                    for h in (0, 1):
                        u = 2 * p + h
                        ot = outp.tile([128, FPAIR], BF, tag="ot")
                        for k in range(NKW):
                            ps = pswp.tile([128, KT], FP, tag="psw")
                            nc.tensor.matmul(
                                ps[:], wmbf[:, 128 * h:128 * (h + 1)],
                                xall[:, u, KT * k:KT * (k + 1)],
                                start=True, stop=True)
                            osl = ot[:, KT * k:KT * (k + 1)]
                            if k % 2 == 0:
                                nc.scalar.activation(
                                    osl, ps[:], AF.Identity,
                                    bias=shift[:, h:h + 1],
                                    scale=wcol[:, h:h + 1])
                            else:
                                nc.vector.tensor_scalar(
                                    osl, ps[:], wcol[:, h:h + 1],
                                    shift[:, h:h + 1],
                                    op0=mybir.AluOpType.mult,
                                    op1=mybir.AluOpType.add)
                        for i in (0, 1):
                            nc.sync.dma_start(
                                Xn[2 * p + i, 128 * h:128 * (h + 1), :],
                                ot[:, HW * i:HW * (i + 1)])

    nc.compile()
    return nc


_PROGS = {}


def _program():
    if "k" not in _PROGS:
        _PROGS["k"] = _build()
    return _PROGS["k"]


def _const_inputs(weight, bias):
    eyebf = np.eye(128, dtype=ml_dtypes.bfloat16)
    eyef = np.eye(128, dtype=np.float32)
    mask = np.kron(np.eye(8, dtype=np.float32),
                   np.ones((CG, CG), dtype=np.float32))
    epseye = (EPS * np.eye(128)).astype(np.float32)
    eye15 = (1.5 * np.eye(128)).astype(np.float32)
    wcol = np.ascontiguousarray(weight.reshape(2, 128).T.astype(np.float32))
    bcol = np.ascontiguousarray(bias.reshape(2, 128).T.astype(np.float32))
    return {"eyebf": eyebf, "eyef": eyef, "mask": mask, "epseye": epseye,
            "eye15": eye15, "wcol": wcol, "bcol": bcol}


def kernel(X, weight, bias, _return_results=False):
    X = np.asarray(X, dtype=np.float32)
    weight = np.asarray(weight, dtype=np.float32).reshape(C)
    bias = np.asarray(bias, dtype=np.float32).reshape(C)
    nc = _program()

    Xr = X.reshape(N, C, HW)
    consts = _const_inputs(weight, bias)
    in_maps = [{"X": Xr[NL * i:NL * (i + 1)].astype(ml_dtypes.bfloat16),
                **consts} for i in range(N_CORES)]

    res = run_bass_kernel_spmd(nc, in_maps, list(range(N_CORES)))

    out = np.concatenate([r["Xn"].astype(np.float32) for r in res.results],
                         axis=0)
    out = out.reshape(N, C, H, W)
    if _return_results:
        return out, res
    return out


# revision 11
# speedup vs baseline: 1.2421x; 1.0410x over previous
"""DBN-Sigma whitening (group-wise decorrelated batch norm) on 8 trn2 cores.

Single-pass design (data-parallel over batch N, hint-conformant):
  The per-core shard (8 of 64 images) is cast to bf16 on the host and
  streamed ONCE into SBUF where it stays resident (98 KiB/partition).
  While loading, each (image-pair, channel-half) unit contributes
  per-channel row sums (ACT accum) and the two 128x128 diagonal blocks
  of the raw second moment, accumulated in PSUM via PE transposes +
  bf16 matmuls (software-pipelined in batches of 12/13 chunks so the
  DVE PSUM->SBUF bounce hides under the next batch's transposes).
  The stats compact to [128, 34] f32 (per-group 16x16 cov rows + row
  sums, 17 KB) and are AllGather'd across the 8 cores through DRAM
  bounce buffers, then summed on device.
  sigma = S2/M - mu mu^T + eps I (16x16 group blocks) is
  inverted-square-rooted ON DEVICE with 3 coupled Newton-Schulz
  iterations (f32 PE matmuls; sigma ~ I so NS converges quadratically).
  weight folds into the whiten stationary (wm diag(w), via per-partition
  scale + PE transpose); mean/bias fold into a per-channel shift, so
  PSUM evacuation is a single fused add spread over ACT/DVE/GpSimd.
  Whitening runs out of the resident SBUF copy (bf16 matmuls) and the
  bf16 output is upcast to f32 on the host.

HBM traffic per core: 12.85 MB in + 12.85 MB out (vs 64 MB for the
two-pass f32 baseline).
"""

import numpy as np
import ml_dtypes
import concourse.bass as bass
import concourse.bacc as bacc
import concourse.mybir as mybir
import concourse.tile as tile
from concourse.bass_utils import run_bass_kernel_spmd

N_CORES = 8
N, C, H, W = 64, 256, 56, 56
HW = H * W                     # 3136
NL = N // N_CORES              # 8 images per core
G, CG = 16, 16
EPS = 1e-3
M_TOT = N * HW                 # 200704
FP = mybir.dt.float32
BF = mybir.dt.bfloat16

NP_ = NL // 2                  # 4 image pairs per core
NU = 2 * NP_                   # 8 (pair, half) units
FPAIR = 2 * HW                 # 6272 free elems per (pair, half)
NCH = FPAIR // 128             # 49 m-chunks per unit
BATCHES = (12, 12, 12, 13)     # chunk batching for the transpose pipeline
KT = 448                       # whiten matmul free-dim tile (14 * 448 = 6272)
NKW = FPAIR // KT              # 14
NS_ITERS = 3
AF = mybir.ActivationFunctionType
ALU = mybir.AluOpType
# PSUM evacuation engine per whiten chunk (cycled): balance ACT/DVE/GpSimd
EVAC = ("act", "vec") * 7


def _build():
    nc = bacc.Bacc("TRN2", target_bir_lowering=False, debug=False,
                   num_devices=N_CORES)
    X_d = nc.dram_tensor("X", [NL, C, HW], BF, kind="ExternalInput")
    eyebf_d = nc.dram_tensor("eyebf", [128, 128], BF, kind="ExternalInput")
    eyef_d = nc.dram_tensor("eyef", [128, 128], FP, kind="ExternalInput")
    mask_d = nc.dram_tensor("mask", [128, 128], FP, kind="ExternalInput")
    eye15_d = nc.dram_tensor("eye15", [128, 128], FP, kind="ExternalInput")
    epsc_d = nc.dram_tensor("epsc", [128, 34], FP, kind="ExternalInput")
    w_d = nc.dram_tensor("wcol", [128, 2], FP, kind="ExternalInput")
    b_d = nc.dram_tensor("bcol", [128, 2], FP, kind="ExternalInput")
    Xn_d = nc.dram_tensor("Xn", [NL, C, HW], BF, kind="ExternalOutput")
    X = X_d.ap()
    Xn = Xn_d.ap()

    with tile.TileContext(nc) as tc:
        with (
            tc.tile_pool(name="const", bufs=1) as constp,
            tc.tile_pool(name="xres", bufs=1) as xresp,
            tc.tile_pool(name="scr", bufs=2) as scrp,
            tc.tile_pool(name="xtq", bufs=3) as xtqp,
            tc.tile_pool(name="stats", bufs=1) as statp,
            tc.tile_pool(name="small", bufs=1) as smallp,
            tc.tile_pool(name="out", bufs=3) as outp,
            tc.tile_pool(name="dram", bufs=1, space="DRAM") as dramp,
        ):
            eyebf = constp.tile([128, 128], BF, tag="eyebf")
            nc.sync.dma_start(eyebf[:], eyebf_d.ap())
            eyef = constp.tile([128, 128], FP, tag="eyef")
            nc.sync.dma_start(eyef[:], eyef_d.ap())
            mask = constp.tile([128, 128], FP, tag="mask")
            nc.sync.dma_start(mask[:], mask_d.ap())
            eye15 = constp.tile([128, 128], FP, tag="eye15")
            nc.sync.dma_start(eye15[:], eye15_d.ap())
            epsc = constp.tile([128, 34], FP, tag="epsc")
            nc.sync.dma_start(epsc[:], epsc_d.ap())
            wcol = constp.tile([128, 2], FP, tag="wcol")
            nc.sync.dma_start(wcol[:], w_d.ap())
            bcol = constp.tile([128, 2], FP, tag="bcol")
            nc.sync.dma_start(bcol[:], b_d.ap())

            xall = xresp.tile([128, NU, FPAIR], BF)       # resident shard
            rs = statp.tile([128, 2, NP_], FP)            # per-unit row sums
            cmp_sb = statp.tile([128, 34], FP)            # compact local stats
            gth = statp.tile([128, N_CORES, 34], FP)      # gathered stats
            red2 = statp.tile([128, 34], FP)              # summed / M + eps

            # ---- phase 1: load resident + stats -------------------------
            # software-pipelined PE program:  T(b) ; [T(b+1)] ; mm(b) ; ...
            with (
                tc.tile_pool(name="ptp", bufs=2, space="PSUM") as ptp,
                tc.tile_pool(name="covp", bufs=1, space="PSUM") as covp,
            ):
                cov = [covp.tile([128, 128], FP, tag=f"cov{h}", name=f"cov{h}")
                       for h in (0, 1)]
                started = [False, False]
                pending = None          # (h, xtq_tile, nch, last_of_cov)

                def flush_pending():
                    nonlocal pending
                    if pending is None:
                        return
                    fh, fx, fn, flast = pending
                    for jj in range(fn):
                        nc.tensor.matmul(
                            cov[fh][:], fx[:, 128 * jj:128 * (jj + 1)],
                            fx[:, 128 * jj:128 * (jj + 1)],
                            start=not started[fh],
                            stop=(flast and jj == fn - 1),
                            skip_group_check=True)
                        started[fh] = True
                    pending = None

                for p in range(NP_):
                    for h in (0, 1):
                        u = 2 * p + h
                        for i in (0, 1):
                            nc.sync.dma_start(
                                xall[:, u, HW * i:HW * (i + 1)],
                                X[2 * p + i, 128 * h:128 * (h + 1), :])
                        xu = xall[:, u, :]
                        scr = scrp.tile([128, FPAIR], BF, tag="scr")
                        nc.scalar.activation(scr[:], xu, AF.Copy,
                                             accum_out=rs[:, h, p:p + 1])
                        last_u = (p == NP_ - 1)
                        c0 = 0
                        for bi, nch in enumerate(BATCHES):
                            pt = ptp.tile([128, nch * 128], BF, tag="pt")
                            for jj in range(nch):
                                m0 = 128 * (c0 + jj)
                                nc.tensor.transpose(
                                    pt[:, 128 * jj:128 * (jj + 1)],
                                    xall[:, u, m0:m0 + 128], eyebf[:])
                            flush_pending()
                            xtq = xtqp.tile([128, nch * 128], BF, tag="xtq")
                            nc.vector.tensor_copy(xtq[:], pt[:])
                            pending = (h, xtq, nch,
                                       last_u and bi == len(BATCHES) - 1)
                            c0 += nch
                flush_pending()

                # compact: cmp[p, 16h+j] = sum_o (cov[h] * mask)[p, 16o+j]
                # (mask keeps only each row's own 16x16 block, so the sum
                #  over block-columns just picks out that block's entries)
                for h in (0, 1):
                    cm3 = statp.tile([128, 8, 16], FP, tag=f"cm3{h}",
                                     name=f"cm3{h}")
                    for o in range(8):
                        sl = slice(16 * o, 16 * (o + 1))
                        nc.vector.tensor_mul(cm3[:, o, :], cov[h][:, sl],
                                             mask[:, sl])
                    nc.vector.tensor_reduce(cmp_sb[:, 16 * h:16 * (h + 1)],
                                            cm3[:].transpose([0, 2, 1]),
                                            axis=mybir.AxisListType.X,
                                            op=ALU.add)
                    nc.vector.tensor_reduce(cmp_sb[:, 32 + h:33 + h],
                                            rs[:, h, :],
                                            axis=mybir.AxisListType.X,
                                            op=ALU.add)

            # ---- collective: gather stats from the 8 cores --------------
            st_in = dramp.tile([128, 34], FP, tag="st_in")
            st_out = dramp.tile([N_CORES, 128, 34], FP, tag="st_out")
            nc.sync.dma_start(st_in[:], cmp_sb[:])
            nc.gpsimd.collective_compute(
                "AllGather", ALU.bypass,
                replica_groups=[list(range(N_CORES))],
                ins=[st_in.opt()], outs=[st_out.opt()])
            nc.sync.dma_start(gth[:], st_out[:].transpose([1, 0, 2]))

            # red2 = (sum_cores stats) / M + eps-on-diag (compact layout)
            nc.vector.tensor_reduce(red2[:], gth[:].transpose([0, 2, 1]),
                                    axis=mybir.AxisListType.X, op=ALU.add)
            nc.vector.tensor_scalar(red2[:], red2[:], 1.0 / M_TOT, None,
                                    op0=ALU.mult)
            nc.vector.tensor_add(red2[:], red2[:], epsc[:])

            # ---- phase 2: sigma -> wm = sigma^(-1/2) on device ----------
            wmbf = smallp.tile([128, 256], BF, tag="wmbf")
            shift = smallp.tile([128, 2], FP, tag="shift")
            with tc.tile_pool(name="ps2", bufs=2, space="PSUM") as ps2p:
                meanc = red2[:, 32:34]    # already includes the /M
                Yt = [smallp.tile([128, 128], FP, tag=f"Y{h}", name=f"Y{h}")
                      for h in (0, 1)]
                Zt = [smallp.tile([128, 128], FP, tag=f"Z{h}", name=f"Z{h}")
                      for h in (0, 1)]
                Tt = [smallp.tile([128, 128], FP, tag=f"T{h}", name=f"T{h}")
                      for h in (0, 1)]
                for h in (0, 1):
                    # mean row (for the outer product) via PE transpose
                    s1pad = smallp.tile([128, 128], FP, tag=f"s1pad{h}",
                                        name=f"s1pad{h}")
                    nc.vector.memset(s1pad[:], 0.0)
                    nc.vector.tensor_copy(s1pad[:, 0:1], meanc[:, h:h + 1])
                    psT = ps2p.tile([128, 128], FP, tag="ps2s")
                    nc.tensor.transpose(psT[:], s1pad[:], eyef[:])
                    mT = smallp.tile([128, 128], FP, tag=f"meanT{h}",
                                     name=f"meanT{h}")
                    nc.scalar.activation(mT[0:1, :], psT[0:1, :], AF.Copy)
                    # sigma: expand compact blocks (stripe-wise mask mult),
                    # then subtract the masked mu mu^T outer product
                    for o in range(8):
                        sl = slice(16 * o, 16 * (o + 1))
                        eng = nc.gpsimd if o % 2 == 0 else nc.vector
                        eng.tensor_mul(Yt[h][:, sl],
                                       red2[:, 16 * h:16 * (h + 1)],
                                       mask[:, sl])
                    po = ps2p.tile([128, 128], FP, tag="ps2s")
                    nc.tensor.matmul(po[:], mT[0:1, :], mT[0:1, :],
                                     start=True, stop=True)
                    pom = smallp.tile([128, 128], FP, tag=f"pom{h}",
                                      name=f"pom{h}")
                    nc.vector.tensor_mul(pom[:], po[:], mask[:])
                    nc.vector.tensor_sub(Yt[h][:], Yt[h][:], pom[:])
                    nc.gpsimd.tensor_copy(Zt[h][:], eyef[:])
                # Newton-Schulz: T = 1.5 I - 0.5 Z Y ; Y <- Y T ; Z <- T Z
                # (iterates are polynomials in sigma -> symmetric, so
                #  matmul's lhsT-transpose is a no-op)
                for it in range(NS_ITERS):
                    for h in (0, 1):
                        pzy = ps2p.tile([128, 128], FP, tag="ps2s")
                        nc.tensor.matmul(pzy[:], Zt[h][:], Yt[h][:],
                                         start=True, stop=True)
                        nc.vector.scalar_tensor_tensor(
                            Tt[h][:], pzy[:], -0.5, eye15[:],
                            op0=ALU.mult, op1=ALU.add)
                        if it < NS_ITERS - 1:
                            pyy = ps2p.tile([128, 128], FP, tag="ps2s")
                            nc.tensor.matmul(pyy[:], Yt[h][:], Tt[h][:],
                                             start=True, stop=True)
                            nc.scalar.activation(Yt[h][:], pyy[:], AF.Copy)
                        pzz = ps2p.tile([128, 128], FP, tag="ps2s")
                        nc.tensor.matmul(pzz[:], Tt[h][:], Zt[h][:],
                                         start=True, stop=True)
                        nc.vector.tensor_copy(Zt[h][:], pzz[:])
                # fold weight into the stationary: lhsT = (diag(w) wm)^T
                for h in (0, 1):
                    wmw = smallp.tile([128, 128], FP, tag=f"wmw{h}",
                                      name=f"wmw{h}")
                    nc.vector.tensor_scalar(wmw[:], Zt[h][:],
                                            wcol[:, h:h + 1], None,
                                            op0=ALU.mult)
                    pwT = ps2p.tile([128, 128], FP, tag="ps2s")
                    nc.tensor.transpose(pwT[:], wmw[:], eyef[:])
                    nc.vector.tensor_copy(wmbf[:, 128 * h:128 * (h + 1)],
                                          pwT[:])
                    pm = ps2p.tile([128, 1], FP, tag="pm")
                    nc.tensor.matmul(pm[:], Zt[h][:], meanc[:, h:h + 1],
                                     start=True, stop=True)
                    nc.vector.tensor_mul(shift[:, h:h + 1], wcol[:, h:h + 1],
                                        pm[:])
                    nc.vector.tensor_sub(shift[:, h:h + 1], bcol[:, h:h + 1],
                                        shift[:, h:h + 1])

            # ---- phase 3: whiten from resident SBUF ---------------------
            with tc.tile_pool(name="psw", bufs=6, space="PSUM") as pswp:
                for p in range(NP_):
                    for h in (0, 1):
                        u = 2 * p + h
                        ot = outp.tile([128, FPAIR], BF, tag="ot")
                        for k in range(NKW):
                            ps = pswp.tile([128, KT], FP, tag="psw")
                            nc.tensor.matmul(
                                ps[:], wmbf[:, 128 * h:128 * (h + 1)],
                                xall[:, u, KT * k:KT * (k + 1)],
                                start=True, stop=True)
                            osl = ot[:, KT * k:KT * (k + 1)]
                            eng = EVAC[k]
                            if eng == "act":
                                nc.scalar.activation(
                                    osl, ps[:], AF.Identity,
                                    bias=shift[:, h:h + 1], scale=1.0)
                            else:
                                nc.vector.tensor_scalar(
                                    osl, ps[:], shift[:, h:h + 1], None,
                                    op0=ALU.add)
                        for i in (0, 1):
                            nc.sync.dma_start(
                                Xn[2 * p + i, 128 * h:128 * (h + 1), :],
                                ot[:, HW * i:HW * (i + 1)])

    nc.compile()
    return nc


_PROGS = {}


def _program():
    if "k" not in _PROGS:
        _PROGS["k"] = _build()
    return _PROGS["k"]


def _const_inputs(weight, bias):
    eyebf = np.eye(128, dtype=ml_dtypes.bfloat16)
    eyef = np.eye(128, dtype=np.float32)
    mask = np.kron(np.eye(8, dtype=np.float32),
                   np.ones((CG, CG), dtype=np.float32))
    eye15 = (1.5 * np.eye(128)).astype(np.float32)
    epsc = np.zeros((128, 34), dtype=np.float32)
    for p in range(128):
        epsc[p, p % 16] = EPS
        epsc[p, 16 + p % 16] = EPS
    wcol = np.ascontiguousarray(weight.reshape(2, 128).T.astype(np.float32))
    bcol = np.ascontiguousarray(bias.reshape(2, 128).T.astype(np.float32))
    return {"eyebf": eyebf, "eyef": eyef, "mask": mask, "eye15": eye15,
            "epsc": epsc, "wcol": wcol, "bcol": bcol}


def kernel(X, weight, bias, _return_results=False):
    X = np.asarray(X, dtype=np.float32)
    weight = np.asarray(weight, dtype=np.float32).reshape(C)
    bias = np.asarray(bias, dtype=np.float32).reshape(C)
    nc = _program()

    Xr = X.reshape(N, C, HW)
    consts = _const_inputs(weight, bias)
    in_maps = [{"X": Xr[NL * i:NL * (i + 1)].astype(ml_dtypes.bfloat16),
                **consts} for i in range(N_CORES)]

    res = run_bass_kernel_spmd(nc, in_maps, list(range(N_CORES)))

    out = np.concatenate([r["Xn"].astype(np.float32) for r in res.results],
                         axis=0)
    out = out.reshape(N, C, H, W)
    if _return_results:
        return out, res
    return out
